# revision 33
# baseline (speedup 1.0000x reference)
import sys

if "/opt/trn_rl_repo" not in sys.path:
    sys.path.insert(0, "/opt/trn_rl_repo")

import numpy as np

# nn_PolylineSubgraphEncoder: 2-layer GCN, N=50000 nodes, E=800000 edges.
N = 50000
E = 800000
H = 64
IN = 4
P = 128
CORES = 8
WPC = 49                 # windows per core (1 window = 128 dest slots)
NW = CORES * WPC         # 392 global windows
NPC = WPC * P            # 6272 dests per core
NPAD = NW * P            # 50176
SPLIT = 32768            # int16 gather-index split
ROWS1 = P * (NW + 1)     # 50304 g1 table rows (col NW is zeros)
ROWS2 = CORES * P * WPC + 2 * P  # 50432 g2f rows: [128 zeros|batches|128 zeros]
PAD1_LO = NW             # row (p=0, w=392): zeros
PAD1_HI = ROWS1 - 1 - SPLIT    # 17535
PAD2_LO = 0              # leading zero block
PAD2_HI = ROWS2 - 1 - SPLIT    # 17663 (trailing zero block)
SC_CAP = 160             # max levels (lo+hi) per super-chunk

TRACE = False
LAST_RESULT = None


def _wrap_idx(a):
    """int array (len % 2048 == 0) -> SWDGE idx layout [128, len/16] int16."""
    a = np.ascontiguousarray(a.astype(np.int16))
    w = a.reshape(-1, 16).T
    return np.ascontiguousarray(np.tile(w, (8, 1)))


def _edge_levels(dest_keys, nkeys):
    """Per-edge rank j within its dest_key group (stable order)."""
    order = np.argsort(dest_keys, kind="stable")
    ks = dest_keys[order]
    starts = np.r_[0, np.flatnonzero(ks[1:] != ks[:-1]) + 1]
    lens = np.diff(np.r_[starts, len(ks)])
    j = np.arange(len(ks)) - np.repeat(starts, lens)
    out = np.empty(len(ks), np.int64)
    out[order] = j
    return out


def _layout_layer(srow, d):
    """Choose dest->(core,lw,slot) assignment + positional idx streams.

    srow: per-edge source table row. d: per-edge dest node (padded ids).
    """
    lo = srow < SPLIT
    a = np.bincount(d[lo], minlength=NPAD)
    b = np.bincount(d[~lo], minlength=NPAD)
    bkey = np.where(a % 2 == 0, b, b.max() - b)  # snake within a-strata
    order = np.lexsort((bkey, a))
    pos = np.empty(NPAD, np.int64)
    pos[order] = np.arange(NPAD)
    lw_of = pos // 1024
    k = pos % 1024
    c_of = k // P
    slot_of = k % P
    Llo = a[order].reshape(WPC, 1024).max(1)
    Lhi = b[order].reshape(WPC, 1024).max(1)
    cumlo = np.r_[0, np.cumsum(Llo)]
    cumhi = np.r_[0, np.cumsum(Lhi)]
    nlo = int(cumlo[-1]) * P
    nhi = int(cumhi[-1]) * P

    j = _edge_levels(d * 2 + (~lo).astype(np.int64), NPAD * 2)
    dc, dlw, dslot = c_of[d], lw_of[d], slot_of[d]

    streams_lo = [np.full(nlo, -1, np.int64) for _ in range(CORES)]
    streams_hi = [np.full(nhi, -1, np.int64) for _ in range(CORES)]
    for c in range(CORES):
        m = (dc == c) & lo
        posn = (cumlo[dlw[m]] + j[m]) * P + dslot[m]
        streams_lo[c][posn] = srow[m]
        m = (dc == c) & ~lo
        posn = (cumhi[dlw[m]] + j[m]) * P + dslot[m]
        streams_hi[c][posn] = srow[m] - SPLIT

    # super-chunks: consecutive windows, sum(Llo+Lhi) <= SC_CAP
    scs = []
    wb = 0
    while wb < WPC:
        wn = 1
        while wb + wn < WPC and (cumlo[wb + wn + 1] - cumlo[wb]) + (
            cumhi[wb + wn + 1] - cumhi[wb]
        ) <= SC_CAP:
            wn += 1
        scs.append((wb, wn))
        wb += wn

    node_at = np.empty((CORES, WPC, P), np.int64)
    node_at[c_of[order], lw_of[order], slot_of[order]] = order

    return dict(
        Llo=Llo, Lhi=Lhi, cumlo=cumlo, cumhi=cumhi, nlo=nlo, nhi=nhi,
        scs=scs, node_at=node_at, c_of=c_of, lw_of=lw_of, slot_of=slot_of,
        streams_lo=streams_lo, streams_hi=streams_hi,
    )


def preprocess(x, edge_index):
    x = np.asarray(x, dtype=np.float32)
    ei = np.asarray(edge_index)
    src = ei[0].astype(np.int64)
    dst = ei[1].astype(np.int64)
    loop = np.arange(N, dtype=np.int64)
    s = np.concatenate([src, loop])
    d = np.concatenate([dst, loop])

    deg = np.bincount(d, minlength=N).astype(np.float32)
    dinv = np.zeros(NPAD, np.float32)
    dinv[:N] = 1.0 / np.sqrt(deg)

    xsT = np.zeros((IN, NPAD), np.float32)
    xsT[:, :N] = (x * dinv[:N, None]).T

    row1_of = (np.arange(NPAD) & 127) * (NW + 1) + (np.arange(NPAD) >> 7)
    L1 = _layout_layer(row1_of[s], d)
    L1["pad_lo"], L1["pad_hi"] = PAD1_LO, PAD1_HI

    # collective batches: groups of L1 super-chunks (>=16 windows each) so
    # partial AllGathers overlap later L1 compute; g2f is batch-major
    bounds = []
    if PIPE_CC:
        acc = 0
        for _wb, _wn in L1["scs"]:
            acc += _wn
            if acc - (bounds[-1] if bounds else 0) >= 16:
                bounds.append(acc)
    if not bounds or bounds[-1] != WPC:
        bounds.append(WPC)
    w0_of = np.empty(WPC, np.int64)
    bn_of = np.empty(WPC, np.int64)
    prev = 0
    for b in bounds:
        w0_of[prev:b] = prev
        bn_of[prev:b] = b - prev
        prev = b

    # g2f row of node v (as L2 source) from its L1 placement:
    # batch-major AllGather layout + one zero block at each end
    lw, c_, sl = L1["lw_of"], L1["c_of"], L1["slot_of"]
    row2_of = (P + CORES * P * w0_of[lw] + c_ * bn_of[lw] * P
               + (lw - w0_of[lw]) * P + sl)
    L2 = _layout_layer(row2_of[s], d)
    L2["pad_lo"], L2["pad_hi"] = PAD2_LO, PAD2_HI

    for L in (L1, L2):
        pl, ph = L["pad_lo"], L["pad_hi"]
        L["ilo"] = [
            _wrap_idx(np.where(st < 0, pl, st)) for st in L["streams_lo"]
        ]
        L["ihi"] = [
            _wrap_idx(np.where(st < 0, ph, st)) for st in L["streams_hi"]
        ]

    cores = []
    for c in range(CORES):
        dinv1w = dinv[L1["node_at"][c]].T  # [P, WPC] (slot, lw)
        dinv2w = dinv[L2["node_at"][c]].T
        cores.append(
            dict(
                dinv1w=np.ascontiguousarray(dinv1w.astype(np.float32)),
                dinv2w=np.ascontiguousarray(dinv2w.astype(np.float32)),
            )
        )
    return dict(xsT=xsT, L1=L1, L2=L2, cores=cores, dinv=dinv,
                cc_bounds=bounds, w0_of=w0_of, bn_of=bn_of)


MAXLEV = 8  # 1024 indices per dma_gather (SWDGE descriptor-ring limit)
# SWDGE queues; chunks round-robin so desc-gen overlaps drain
NQUEUES = int(__import__("os").environ.get("KERNEL_NQ", "4"))
PIPE_CC = int(__import__("os").environ.get("KERNEL_PIPECC", "0")) != 0
SKIP_GATHERS = False  # timing experiments only
_gq = [0]  # round-robin queue counter, reset per build


def _chunked_gather(nc, dst, tab, idx_sb, lev0, nlev):
    """dst[:, 0:nlev, :] = gather of stream levels [lev0, lev0+nlev),
    split into <=MAXLEV-level calls to fit the runtime's descriptor ring."""
    if SKIP_GATHERS:
        return
    for a in range(0, nlev, MAXLEV):
        b = min(a + MAXLEV, nlev)
        nc.gpsimd.dma_gather(
            dst[:, a:b, :], tab,
            idx_sb[:, (lev0 + a) * 8 : (lev0 + b) * 8],
            num_idxs=(b - a) * P, num_idxs_reg=(b - a) * P, elem_size=H,
            queue_num=_gq[0] % NQUEUES,
        )
        _gq[0] += 1


def _gather_layer(nc, gl, lo_tab, hi_tab, ilo_sb, ihi_sb, gpool, epilogue):
    """Positional gathers + per-window free-dim reduces for one layer."""
    from concourse import mybir

    f32 = mybir.dt.float32
    Llo, Lhi = gl["Llo"], gl["Lhi"]
    cumlo, cumhi = gl["cumlo"], gl["cumhi"]
    for wb, wn in gl["scs"]:
        nlo_sc = int(cumlo[wb + wn] - cumlo[wb])
        nhi_sc = int(cumhi[wb + wn] - cumhi[wb])
        gtl = gpool.tile([P, max(nlo_sc, 1), H], f32, name="gtl", tag="gtl")
        gth = gpool.tile([P, max(nhi_sc, 1), H], f32, name="gth", tag="gth")
        if nlo_sc:
            _chunked_gather(nc, gtl, lo_tab, ilo_sb, int(cumlo[wb]), nlo_sc)
        if nhi_sc:
            _chunked_gather(nc, gth, hi_tab, ihi_sb, int(cumhi[wb]), nhi_sc)
        epilogue.begin_sc(wb, wn)
        for wi in range(wn):
            w = wb + wi
            llo, lhi = int(Llo[w]), int(Lhi[w])
            olo = int(cumlo[w] - cumlo[wb])
            ohi = int(cumhi[w] - cumhi[wb])
            epilogue.window(w, wi, gtl, gth, olo, llo, ohi, lhi)
        epilogue.end_sc(wb, wn)


def build_program(pre, debug=False, iters=1, mock_cc=False, parts=None):
    """parts: None = full body; subset of {"A","L1","CC","L2"} to emit only
    those phases (timing experiments; results invalid unless full)."""
    from concourse import bacc, mybir, tile, library_config
    from contextlib import ExitStack

    f32 = mybir.dt.float32
    i16 = mybir.dt.int16
    L1, L2 = pre["L1"], pre["L2"]

    # bacc.Bacc (not bass.Bass): its finalize() runs generate_event_semaphores,
    # which splits multi-wait instructions down to the 1-wait-per-instruction
    # limit of the walrus build in this container.
    _gq[0] = 0
    nc = bacc.Bacc("TRN2", target_bir_lowering=False, debug=debug,
                   num_swdge_queues=NQUEUES)

    xsT_d = nc.declare_dram_parameter("xsT", [IN, NPAD], f32, isOutput=False)
    W1_d = nc.declare_dram_parameter("W1", [IN, H], f32, isOutput=False)
    W2_d = nc.declare_dram_parameter("W2", [H, H], f32, isOutput=False)
    b1bc_d = nc.declare_dram_parameter("b1bc", [P, H], f32, isOutput=False)
    b2bc_d = nc.declare_dram_parameter("b2bc", [P, H], f32, isOutput=False)
    zbc_d = nc.declare_dram_parameter("zbc", [P, H], f32, isOutput=False)
    ident_d = nc.declare_dram_parameter("ident", [P, P], f32, isOutput=False)
    d1w_d = nc.declare_dram_parameter("d1w", [P, WPC], f32, isOutput=False)
    d2w_d = nc.declare_dram_parameter("d2w", [P, WPC], f32, isOutput=False)
    i1lo_d = nc.declare_dram_parameter("i1lo", [P, L1["nlo"] // 16], i16, isOutput=False)
    i1hi_d = nc.declare_dram_parameter("i1hi", [P, L1["nhi"] // 16], i16, isOutput=False)
    i2lo_d = nc.declare_dram_parameter("i2lo", [P, L2["nlo"] // 16], i16, isOutput=False)
    i2hi_d = nc.declare_dram_parameter("i2hi", [P, L2["nhi"] // 16], i16, isOutput=False)
    out_d = nc.declare_dram_parameter("out", [P, WPC, H], f32, isOutput=True)

    g1 = nc.dram_tensor("g1", [P, NW + 1, H], f32)
    g2s = nc.dram_tensor("g2s", [WPC, P, H], f32)  # window-major: batch
    g2f = nc.dram_tensor("g2f", [ROWS2, H], f32, addr_space="Shared")
    cc_bounds = pre["cc_bounds"]

    es = ExitStack()
    with es:
        tc = es.enter_context(tile.TileContext(nc))
        cpool = es.enter_context(tc.tile_pool(name="consts", bufs=1))
        wpool = es.enter_context(tc.tile_pool(name="work", bufs=2))
        gpool = es.enter_context(tc.tile_pool(name="gath", bufs=2))
        psA = es.enter_context(tc.tile_pool(name="psA", bufs=2, space="PSUM"))
        psB = es.enter_context(tc.tile_pool(name="psB", bufs=2, space="PSUM"))

        nc.gpsimd.load_library(library_config.mlp)

        def const(name, shape, dtype, src):
            t = cpool.tile(shape, dtype, name=name, tag=name)
            nc.sync.dma_start(out=t, in_=src)
            return t

        W1_sb = const("W1sb", [IN, H], f32, W1_d[:, :])
        W2_sb = const("W2sb", [H, H], f32, W2_d[:, :])
        b1bc_sb = const("b1bcsb", [P, H], f32, b1bc_d[:, :])
        b2bc_sb = const("b2bcsb", [P, H], f32, b2bc_d[:, :])
        zbc_sb = const("zbcsb", [P, H], f32, zbc_d[:, :])
        id_sb = const("idsb", [P, P], f32, ident_d[:, :])
        d1w_sb = const("d1wsb", [P, WPC], f32, d1w_d[:, :])
        d2w_sb = const("d2wsb", [P, WPC], f32, d2w_d[:, :])
        i1lo_sb = const("i1losb", [P, L1["nlo"] // 16], i16, i1lo_d[:, :])
        i1hi_sb = const("i1hisb", [P, L1["nhi"] // 16], i16, i1hi_d[:, :])
        i2lo_sb = const("i2losb", [P, L2["nlo"] // 16], i16, i2lo_d[:, :])
        i2hi_sb = const("i2hisb", [P, L2["nhi"] // 16], i16, i2hi_d[:, :])

        # zero pad regions of the tables
        nc.sync.dma_start(out=g1[:, NW, :], in_=zbc_sb)
        nc.sync.dma_start(out=g2f[0:P, :], in_=zbc_sb)
        nc.sync.dma_start(out=g2f[ROWS2 - P : ROWS2, :], in_=zbc_sb)

        def phase_a():
            # Phase A (replicated): g1[p, w, :] = (dinv*x)[w*128+p] @ W1.
            # 16 windows per group (2 PSUM banks) to amortize DMA/seq costs.
            GB = 16
            w0 = 0
            while w0 < NW:
                gn = min(GB, NW - w0)
                xsp = wpool.tile([IN, gn * P], f32, name="xsp", tag="xsp")
                nc.sync.dma_start(out=xsp, in_=xsT_d[:, w0 * P : (w0 + gn) * P])
                g1sb = wpool.tile([P, gn * H], f32, name="g1sb", tag="g1sb")
                for h0 in range(0, gn, 8):
                    hn = min(8, gn - h0)
                    ps = psA.tile([P, hn * H], f32, name="ps", tag="psA")
                    for k in range(hn):
                        nc.tensor.matmul(ps[:, k * H : (k + 1) * H],
                                         xsp[:, (h0 + k) * P : (h0 + k + 1) * P],
                                         W1_sb, start=True, stop=True)
                    nc.scalar.copy(g1sb[:, h0 * H : (h0 + hn) * H], ps)
                nc.sync.dma_start(out=g1[:, w0 : w0 + gn, :], in_=g1sb)
                w0 += gn

        g1_flat = g1[:, :, :].flatten_outer_dims()
        g2_flat = g2f[:, :]

        def agg_window(gtl, gth, olo, llo, ohi, lhi):
            """Sum gathered levels -> [P, H] sbuf tile."""
            t = wpool.tile([P, H], f32, name="agg", tag="agg")
            if llo and lhi:
                ta = wpool.tile([P, H], f32, name="ta", tag="ta")
                nc.vector.tensor_reduce(
                    ta, gtl[:, olo : olo + llo, :].transpose([0, 2, 1]),
                    mybir.AxisListType.X, mybir.AluOpType.add)
                tb = wpool.tile([P, H], f32, name="tb", tag="tb")
                nc.vector.tensor_reduce(
                    tb, gth[:, ohi : ohi + lhi, :].transpose([0, 2, 1]),
                    mybir.AxisListType.X, mybir.AluOpType.add)
                nc.vector.tensor_tensor(t, ta, tb, mybir.AluOpType.add)
            elif llo:
                nc.vector.tensor_reduce(
                    t, gtl[:, olo : olo + llo, :].transpose([0, 2, 1]),
                    mybir.AxisListType.X, mybir.AluOpType.add)
            elif lhi:
                nc.vector.tensor_reduce(
                    t, gth[:, ohi : ohi + lhi, :].transpose([0, 2, 1]),
                    mybir.AxisListType.X, mybir.AluOpType.add)
            else:
                nc.scalar.copy(t, zbc_sb)
            return t

        # pipelined collective: partial AllGathers over the cc batches from
        # preprocess (window-major g2s keeps every collective AP contiguous)
        _enabled = parts if parts is not None else {"A", "L1", "CC", "L2"}
        inline_cc = (len(cc_bounds) > 1 and not mock_cc
                     and {"L1", "CC"} <= _enabled)

        def emit_cc(w0, w1):
            nc.gpsimd.collective_compute(
                "AllGather", mybir.AluOpType.bypass,
                replica_groups=[list(range(CORES))],
                ins=[g2s[w0:w1, :, :]],
                outs=[g2f[P + CORES * P * w0 : P + CORES * P * w1, :]],
            )

        class L1Epi:
            def begin_sc(self, wb, wn):
                pass

            def window(self, w, wi, gtl, gth, olo, llo, ohi, lhi):
                agg = agg_window(gtl, gth, olo, llo, ohi, lhi)
                dv = d1w_sb[:, w : w + 1]
                t2 = wpool.tile([P, H], f32, name="t2", tag="t2")
                nc.scalar.activation(t2, agg, mybir.ActivationFunctionType.Copy,
                                     scale=dv)
                t3 = wpool.tile([P, H], f32, name="t3", tag="t3")
                nc.vector.tensor_tensor(t3, t2, b1bc_sb, mybir.AluOpType.add)
                t4 = wpool.tile([P, H], f32, name="t4", tag="t4")
                nc.scalar.activation(t4, t3, mybir.ActivationFunctionType.Relu)
                t5 = wpool.tile([P, H], f32, name="t5", tag="t5")
                nc.scalar.activation(t5, t4, mybir.ActivationFunctionType.Copy,
                                     scale=dv)
                pT = psB.tile([H, P], f32, name="pT", tag="pT",
                              padded_shape=[P, 512])
                nc.tensor.matmul(pT, t5, id_sb, start=True, stop=True)
                t5T = wpool.tile([H, P], f32, name="t5T", tag="t5T")
                nc.scalar.copy(t5T, pT)
                pg = psB.tile([P, H], f32, name="pg", tag="pg",
                              padded_shape=[P, 512])
                nc.tensor.matmul(pg, t5T, W2_sb, start=True, stop=True)
                g2w = wpool.tile([P, H], f32, name="g2w", tag="g2w")
                nc.scalar.copy(g2w, pg)
                nc.sync.dma_start(out=g2s[w, :, :], in_=g2w)

            def end_sc(self, wb, wn):
                if inline_cc and (wb + wn) in cc_bounds:
                    k = cc_bounds.index(wb + wn)
                    emit_cc(cc_bounds[k - 1] if k else 0, wb + wn)

        def layer1():
            _gather_layer(nc, L1, g1_flat[0:SPLIT, :], g1_flat[SPLIT:ROWS1, :],
                          i1lo_sb, i1hi_sb, gpool, L1Epi())

        def cc():
            if mock_cc:
                for c in range(CORES):
                    nc.sync.dma_start(
                        out=g2f[P + c * WPC * P : P + (c + 1) * WPC * P, :],
                        in_=g2s[:, :, :].flatten_outer_dims())
            elif not inline_cc:
                emit_cc(0, WPC)

        class L2Epi:
            def begin_sc(self, wb, wn):
                self.osb = wpool.tile([P, wn * H], f32, name="osb", tag="osb")

            def window(self, w, wi, gtl, gth, olo, llo, ohi, lhi):
                agg = agg_window(gtl, gth, olo, llo, ohi, lhi)
                dv = d2w_sb[:, w : w + 1]
                t2 = wpool.tile([P, H], f32, name="u2", tag="u2")
                nc.scalar.activation(t2, agg, mybir.ActivationFunctionType.Copy,
                                     scale=dv)
                t3 = wpool.tile([P, H], f32, name="u3", tag="u3")
                nc.vector.tensor_tensor(t3, t2, b2bc_sb, mybir.AluOpType.add)
                nc.scalar.activation(self.osb[:, wi * H : (wi + 1) * H], t3,
                                     mybir.ActivationFunctionType.Relu)

            def end_sc(self, wb, wn):
                nc.sync.dma_start(out=out_d[:, wb : wb + wn, :], in_=self.osb)

        def layer2():
            _gather_layer(nc, L2, g2_flat[0:SPLIT, :], g2_flat[SPLIT:ROWS2, :],
                          i2lo_sb, i2hi_sb, gpool, L2Epi())

        enabled = parts if parts is not None else {"A", "L1", "CC", "L2"}
        for _it in range(iters):
            if "A" in enabled:
                phase_a()
            if "L1" in enabled:
                layer1()
            if "CC" in enabled:
                cc()
            if "L2" in enabled:
                layer2()

    nc.finalize()
    return nc


def make_in_maps(pre, W1, b1, W2, b2):
    W1 = np.ascontiguousarray(np.asarray(W1, np.float32))
    W2 = np.ascontiguousarray(np.asarray(W2, np.float32))
    b1bc = np.ascontiguousarray(
        np.broadcast_to(np.asarray(b1, np.float32)[None, :], (P, H)))
    b2bc = np.ascontiguousarray(
        np.broadcast_to(np.asarray(b2, np.float32)[None, :], (P, H)))
    zbc = np.zeros((P, H), np.float32)
    ident = np.eye(P, dtype=np.float32)
    L1, L2 = pre["L1"], pre["L2"]
    in_maps = []
    for c in range(CORES):
        cc = pre["cores"][c]
        in_maps.append(
            dict(
                xsT=pre["xsT"], W1=W1, W2=W2, b1bc=b1bc, b2bc=b2bc,
                zbc=zbc, ident=ident, d1w=cc["dinv1w"], d2w=cc["dinv2w"],
                i1lo=L1["ilo"][c], i1hi=L1["ihi"][c],
                i2lo=L2["ilo"][c], i2hi=L2["ihi"][c],
            )
        )
    return in_maps


def assemble_output(pre, outs):
    """outs: per-core [128, 49, 64] -> [N, 64] via L2 dest placement."""
    node_at = pre["L2"]["node_at"]  # [CORES, WPC, P]
    full = np.zeros((NPAD, H), np.float32)
    for c in range(CORES):
        full[node_at[c].transpose(1, 0)] = outs[c]  # [P, WPC] nodes
    return np.ascontiguousarray(full[:N])


def kernel_bass(x, edge_index, W1, b1, W2, b2):
    global LAST_RESULT
    from concourse import bass_utils

    pre = preprocess(x, edge_index)
    nc = build_program(pre, debug=False)
    in_maps = make_in_maps(pre, W1, b1, W2, b2)
    res = bass_utils.run_bass_kernel_spmd(
        nc, in_maps, list(range(CORES)), trace=False
    )
    LAST_RESULT = res
    return assemble_output(pre, [r["out"] for r in res.results])


def kernel_numpy(x, edge_index, W1, b1, W2, b2):
    x = np.asarray(x, np.float32)
    ei = np.asarray(edge_index)
    src = ei[0].astype(np.int64)
    dst = ei[1].astype(np.int64)
    n = x.shape[0]
    # self-loops make deg = in_degree + 1 > 0, and contribute a pure
    # diagonal dinv[i]^2 * g[i] that we apply as a vector multiply.
    deg = (np.bincount(dst, minlength=n) + 1).astype(np.float32)
    dinv = (1.0 / np.sqrt(deg)).astype(np.float32)
    norm = (dinv[src] * dinv[dst]).astype(np.float32)
    diag = (dinv * dinv)[:, None]

    try:
        import scipy.sparse as sp

        A = sp.csr_matrix((norm, (dst, src)), shape=(n, n), dtype=np.float32)

        def agg(g):
            out = A @ g
            out += diag * g
            return out

    except Exception:

        def agg(g):
            msg = g[src] * norm[:, None]
            out = np.empty((n, g.shape[1]), np.float32)
            for j in range(g.shape[1]):
                out[:, j] = np.bincount(dst, weights=msg[:, j], minlength=n)
            out += diag * g
            return out

    W1 = np.asarray(W1, np.float32)
    b1 = np.asarray(b1, np.float32)
    W2 = np.asarray(W2, np.float32)
    b2 = np.asarray(b2, np.float32)
    # agg is linear, so aggregate the 4-col x before the dense matmul:
    # agg(x @ W1) == agg(x) @ W1, a 16x cheaper SpMM.
    h = agg(x) @ W1
    h += b1
    np.maximum(h, 0.0, out=h)
    out = agg(h @ W2)
    out += b2
    np.maximum(out, 0.0, out=out)
    return out


def kernel(x, edge_index, W1, b1, W2, b2):
    if int(__import__("os").environ.get("KERNEL_NUMPY", "0")):
        return kernel_numpy(x, edge_index, W1, b1, W2, b2)
    return kernel_bass(x, edge_index, W1, b1, W2, b2)

